# revision 1
# baseline (speedup 1.0000x reference)
"""Trainium2 Bass kernel for ComputeAlignmentError.

reference math:
    t[b,i,j,k] = dot(coords[b,i] - origin[b,j], E[b,j,k])   (per pred/true)
    out[b,i,j] = sqrt(sum_k (t_pred - t_true)^2 + 1e-8)

Quadratic-form formulation:
    u[i]   = [pred_coords[i] (3), true_coords[i] (3), 1]            (7)
    A[j]   = rows_k [E_pred[j,k] (3), -E_true[j,k] (3), -c[j,k]]    (3x7)
    err2[i,j] = u^T (A^T A) u = sum_{p<=q} m28[i,pq] * G28[j,pq]
    with G28 off-diagonal entries pre-scaled by 2 (symmetric fold), K=28.
    out[i,j]  = sqrt(err2 + 1e-8)

O(n^2) work = one K=28 fp32r matmul (PE) + sqrt (ACT) + DMA out.
Sharding: output rows i split across 8 cores; frame prep replicated.

Layout: frames are DMA'd contiguously, so partition p holds frames
j = 16p + c (c = 0..15).  The permutation is undone for free inside the
PSUM->SBUF copies after the PE transposes (strided dst), so GT / output
are in true j order.
"""

import numpy as np

B = 2            # batches
N = 2048         # n residues
NCORES = 8
RPC = N // NCORES          # rows per core per batch = 256
P = 128                    # partitions
NCH = N // P               # j-chunks per batch = 16
G64 = 2 * B * NCH          # (t, b, c) groups = 64
EPS_NORM = 1e-8
EPS_ERR = 1e-8

# pq28 symmetric packing: block p holds (p,p), (p,p+1) .. (p,6)
OFF = [0]
for _p in range(7):
    OFF.append(OFF[-1] + (7 - _p))     # OFF[p] = start of block p; OFF[7] = 28

_cache = {}


def _build():
    import concourse.bass as bass
    import concourse.bacc as bacc
    import concourse.tile as tile
    import concourse.mybir as mybir
    from concourse.masks import make_identity

    F32 = mybir.dt.float32
    F32R = mybir.dt.float32r
    MUL = mybir.AluOpType.mult
    ADD = mybir.AluOpType.add
    SUB = mybir.AluOpType.subtract

    nc = bacc.Bacc("TRN2", target_bir_lowering=False, debug=False,
                   num_devices=NCORES)

    pc_d = nc.dram_tensor("pc", [B, RPC, 3], F32, kind="ExternalInput")
    tc_d = nc.dram_tensor("tcrd", [B, RPC, 3], F32, kind="ExternalInput")
    pf_d = nc.dram_tensor("pf", [B, N, 3, 3], F32, kind="ExternalInput")
    tf_d = nc.dram_tensor("tf", [B, N, 3, 3], F32, kind="ExternalInput")
    out_d = nc.dram_tensor("out", [B, RPC, N], F32, kind="ExternalOutput")

    def v(tileap, offset_elems, dims):
        """AP view: keep partition dim of `tileap`, custom free dims."""
        return bass.AP(tensor=tileap.tensor,
                       offset=tileap.offset + offset_elems,
                       ap=[tileap.ap[0]] + dims)

    with tile.TileContext(nc) as tc:
        with (
            tc.tile_pool(name="consts", bufs=1) as consts,
            tc.tile_pool(name="prep", bufs=1) as prep,
            tc.tile_pool(name="gkp", bufs=2) as gkp,
            tc.tile_pool(name="itile", bufs=4) as itile,
            tc.tile_pool(name="gt", bufs=1) as gtp,
            tc.tile_pool(name="ps_t", bufs=2, space="PSUM") as ps_t,
            tc.tile_pool(name="ps_mm", bufs=4, space="PSUM") as ps_mm,
            tc.tile_pool(name="outp", bufs=4) as outp,
        ):
            ident = consts.tile([P, P], F32)
            make_identity(nc, ident[:])
            eps_t = consts.tile([P, 1], F32)
            nc.vector.memset(eps_t[:], EPS_ERR)

            # ---- frames: contiguous DMA; partition p <- j = 16p + c
            # F[jp, t, b, c, e]; e = d*3 + pt (pt fastest)
            F = prep.tile([P, 2, B, NCH, 9], F32)
            for t, dram in enumerate((pf_d, tf_d)):
                for b in range(B):
                    src = bass.AP(tensor=dram, offset=b * N * 9,
                                  ap=[[144, P], [1, 144]])
                    nc.sync.dma_start(out=F[:, t, b], in_=src)

            # ---- per-i-tile u & m28 (independent of frames; runs early)
            MT = []
            for b in range(B):
                for r in range(RPC // P):
                    U = itile.tile([P, 7], F32, name=f"u{b}{r}", tag="u")
                    off = (b * RPC + r * P) * 3
                    nc.sync.dma_start(
                        out=U[:, 0:3],
                        in_=bass.AP(tensor=pc_d, offset=off, ap=[[3, P], [1, 3]]))
                    nc.sync.dma_start(
                        out=U[:, 3:6],
                        in_=bass.AP(tensor=tc_d, offset=off, ap=[[3, P], [1, 3]]))
                    nc.vector.memset(U[:, 6:7], 1.0)
                    M28 = itile.tile([P, 28], F32, name=f"m{b}{r}", tag="m")
                    for p in range(7):
                        # diag: u_p^2 ; offdiag: 2*u_p*u_q (symmetric fold)
                        nc.gpsimd.tensor_scalar_mul(
                            M28[:, OFF[p]:OFF[p] + 1], U[:, p:p + 1],
                            U[:, p:p + 1])
                        if p < 6:
                            nc.gpsimd.tensor_scalar(
                                M28[:, OFF[p] + 1:OFF[p + 1]], U[:, p + 1:7],
                                U[:, p:p + 1], 2.0, MUL, MUL)
                    tp_m = ps_t.tile([28, P], F32, name=f"tpm{b}{r}", tag="tpm")
                    nc.tensor.transpose(tp_m[:], M28[:], ident[:])
                    MTt = itile.tile([28, P], F32, name=f"mt{b}{r}", tag="mt")
                    nc.vector.tensor_copy(out=MTt[:], in_=tp_m[:])
                    MT.append(MTt)

            fap = F[:]
            def fpt(pt, extra):
                return v(fap, pt, [[9, G64]] + extra)

            # ---- frame bases (vectorized over g = (t,b,c), 64 groups)
            W1 = prep.tile([P, G64, 3], F32)
            W2 = prep.tile([P, G64, 3], F32)
            nc.vector.tensor_tensor(out=W1[:], in0=fpt(0, [[3, 3]]),
                                    in1=fpt(1, [[3, 3]]), op=SUB)
            nc.vector.tensor_tensor(out=W2[:], in0=fpt(2, [[3, 3]]),
                                    in1=fpt(1, [[3, 3]]), op=SUB)

            def normalize_pair(XA, XB, dsts):
                """dsts: list of (dst_ap,) for XA, XB normalized."""
                SQa = prep.tile([P, G64, 3], F32, name=f"sqa{id(XA) % 97}", tag="sqa")
                SQb = prep.tile([P, G64, 3], F32, name=f"sqb{id(XB) % 97}", tag="sqb")
                SS = prep.tile([P, 2, G64], F32, name=f"ss{id(XA) % 97}", tag="ss")
                nc.scalar.square(SQa[:], XA[:])
                nc.scalar.square(SQb[:], XB[:])
                nc.vector.tensor_reduce(out=SS[:, 0], in_=SQa[:],
                                        axis=mybir.AxisListType.X, op=ADD)
                nc.vector.tensor_reduce(out=SS[:, 1], in_=SQb[:],
                                        axis=mybir.AxisListType.X, op=ADD)
                NRM = prep.tile([P, 2, G64], F32, name=f"nrm{id(XA) % 97}", tag="nrm")
                nc.scalar.sqrt(NRM[:], SS[:])
                RC = prep.tile([P, 2, G64], F32, name=f"rc{id(XA) % 97}", tag="rc")
                nc.vector.tensor_scalar_max(RC[:], NRM[:], EPS_NORM)
                RCP = prep.tile([P, 2, G64], F32, name=f"rcp{id(XA) % 97}", tag="rcp")
                nc.vector.reciprocal(RCP[:], RC[:])
                for idx, X in enumerate((XA, XB)):
                    rcp_b = v(RCP[:], idx * G64, [[1, G64], [0, 3]])
                    nc.vector.tensor_tensor(out=dsts[idx], in0=X[:], in1=rcp_b,
                                            op=MUL)

            W1N = prep.tile([P, G64, 3], F32)
            W2N = prep.tile([P, G64, 3], F32)
            normalize_pair(W1, W2, [W1N[:], W2N[:]])

            S = prep.tile([P, G64, 3], F32)
            D = prep.tile([P, G64, 3], F32)
            nc.vector.tensor_tensor(out=S[:], in0=W1N[:], in1=W2N[:], op=ADD)
            nc.vector.tensor_tensor(out=D[:], in0=W2N[:], in1=W1N[:], op=SUB)

            EE = prep.tile([P, G64, 3, 3], F32)   # (g, k, d)
            e1_dst = v(EE[:], 0, [[9, G64], [1, 3]])
            e2_dst = v(EE[:], 3, [[9, G64], [1, 3]])
            normalize_pair(S, D, [e1_dst, e2_dst])

            # e3 = e1 x e2 (split across gpsimd / DVE)
            TA = prep.tile([P, 3, G64], F32)
            TB = prep.tile([P, 3, G64], F32)
            for x in range(3):
                y, z = (x + 1) % 3, (x + 2) % 3
                nc.gpsimd.tensor_tensor(out=TA[:, x], in0=EE[:, :, 0, y],
                                        in1=EE[:, :, 1, z], op=MUL)
                nc.vector.tensor_tensor(out=TB[:, x], in0=EE[:, :, 0, z],
                                        in1=EE[:, :, 1, y], op=MUL)
            for x in range(3):
                nc.vector.tensor_tensor(out=EE[:, :, 2, x], in0=TA[:, x],
                                        in1=TB[:, x], op=SUB)

            # ---- origin projections; CT = (o_t.E_t) - (o_p.E_p) = -c
            OP = prep.tile([P, G64, 3, 3], F32)
            orig_b = fpt(1, [[0, 3], [3, 3]])
            nc.vector.tensor_tensor(out=OP[:], in0=EE[:], in1=orig_b, op=MUL)
            OC = prep.tile([P, G64, 3], F32)
            nc.vector.tensor_reduce(out=OC[:], in_=OP[:],
                                    axis=mybir.AxisListType.X, op=ADD)
            CT = prep.tile([P, B * NCH, 3], F32)
            nc.vector.tensor_tensor(out=CT[:], in0=OC[:, B * NCH:],
                                    in1=OC[:, :B * NCH], op=SUB)

            # ---- A[bc, k, f7] = [Ep | -Et | -c]
            A = prep.tile([P, B * NCH, 3, 7], F32)
            a_ap = A[:]
            nc.gpsimd.tensor_copy(
                out=v(a_ap, 0, [[21, B * NCH], [7, 3], [1, 3]]),
                in_=EE[:, :B * NCH])
            nc.vector.tensor_scalar_mul(
                v(a_ap, 3, [[21, B * NCH], [7, 3], [1, 3]]),
                EE[:, B * NCH:], -1.0)
            nc.gpsimd.tensor_copy(
                out=v(a_ap, 6, [[21, B * NCH], [7, 3]]), in_=CT[:])

            # ---- per batch: G28, transposes, un-permuting copies, matmuls
            GT = [gtp.tile([28, N], F32, name=f"gt{b}", tag=f"gt{b}")
                  for b in range(B)]
            for b in range(B):
                GK = gkp.tile([P, NCH, 28, 3], F32, name=f"gk{b}", tag="gk")
                gk_ap = GK[:]
                aoff = b * NCH * 21
                for p in range(7):
                    # diag: (p,p)
                    nc.gpsimd.tensor_tensor(
                        out=v(gk_ap, OFF[p] * 3, [[84, NCH], [1, 3]]),
                        in0=v(a_ap, aoff + p, [[21, NCH], [7, 3]]),
                        in1=v(a_ap, aoff + p, [[21, NCH], [7, 3]]), op=MUL)
                    nq = 6 - p
                    if nq:
                        # offdiag: A_p * A_q for q = p+1..6 (x2 folded into m28)
                        nc.vector.tensor_tensor(
                            out=v(gk_ap, (OFF[p] + 1) * 3,
                                  [[84, NCH], [1, 3], [3, nq]]),
                            in0=v(a_ap, aoff + p, [[21, NCH], [7, 3], [0, nq]]),
                            in1=v(a_ap, aoff + p + 1,
                                  [[21, NCH], [7, 3], [1, nq]]),
                            op=MUL)
                G28 = gkp.tile([P, NCH, 28], F32, name=f"g28_{b}", tag="g28")
                nc.vector.tensor_reduce(out=G28[:], in_=GK[:],
                                        axis=mybir.AxisListType.X, op=ADD)

                gt_ap = GT[b][:]
                for t_i in range(4):           # four PSUM tiles of 4 chunks
                    tp4 = ps_t.tile([28, 512], F32, name=f"tp4_{b}{t_i}",
                                    tag="tp4")
                    for k in range(4):
                        nc.tensor.transpose(
                            tp4[:, k * 128:(k + 1) * 128],
                            G28[:, 4 * t_i + k, :], ident[:])
                    # un-permute: GT col j = 16p + (4*t_i + k)
                    src = v(tp4[:], 0, [[128, 4], [1, P]])
                    dst = v(gt_ap, 4 * t_i, [[1, 4], [16, P]])
                    nc.vector.tensor_copy(out=dst, in_=src)

                # matmuls for this batch's two i-tiles
                for r in range(RPC // P):
                    MTt = MT[b * (RPC // P) + r]
                    for ch in range(4):
                        mm = ps_mm.tile([P, 512], F32, name=f"mm{b}{r}{ch}",
                                        tag="mm")
                        nc.tensor.matmul(
                            mm[:], MTt[:],
                            GT[b][:, ch * 512:(ch + 1) * 512],
                            start=True, stop=True)
                        OT = outp.tile([P, 512], F32, name=f"ot{b}{r}{ch}",
                                       tag="ot")
                        nc.scalar.activation(
                            out=OT[:], in_=mm[:],
                            func=mybir.ActivationFunctionType.Sqrt,
                            bias=eps_t[:], scale=1.0)
                        dst = bass.AP(
                            tensor=out_d,
                            offset=(b * RPC + r * P) * N + ch * 512,
                            ap=[[N, P], [1, 512]])
                        nc.sync.dma_start(out=dst, in_=OT[:])

    nc.compile()
    return nc


def _get_nc():
    if "nc" not in _cache:
        _cache["nc"] = _build()
    return _cache["nc"]


def _in_maps(pred_coords, true_coords, pred_frames, true_frames):
    pc = np.ascontiguousarray(pred_coords, dtype=np.float32)
    tcd = np.ascontiguousarray(true_coords, dtype=np.float32)
    pf = np.ascontiguousarray(pred_frames, dtype=np.float32)
    tf = np.ascontiguousarray(true_frames, dtype=np.float32)
    maps = []
    for c in range(NCORES):
        sl = slice(c * RPC, (c + 1) * RPC)
        maps.append({
            "pc": np.ascontiguousarray(pc[:, sl]),
            "tcrd": np.ascontiguousarray(tcd[:, sl]),
            "pf": pf,
            "tf": tf,
        })
    return maps


def _assemble(results):
    full = np.empty((B, N, N), dtype=np.float32)
    for c in range(NCORES):
        full[:, c * RPC:(c + 1) * RPC, :] = results[c]["out"]
    return full


def run_hw(trace=False, **inputs):
    from concourse.bass_utils import run_bass_kernel_spmd
    nc = _get_nc()
    res = run_bass_kernel_spmd(nc, _in_maps(**inputs), list(range(NCORES)),
                               trace=trace)
    return _assemble(res.results), res


def kernel(**inputs):
    out, _ = run_hw(trace=False, **inputs)
    return out



# revision 21
# speedup vs baseline: 1.1430x; 1.1430x over previous
"""Trainium2 Bass kernel for ComputeAlignmentError (v2).

reference math:
    t[b,i,j,k] = dot(coords[b,i] - origin[b,j], E[b,j,k])   (per pred/true)
    out[b,i,j] = sqrt(sum_k (t_pred - t_true)^2 + 1e-8)

K=18 quadratic form exploiting orthonormality of the frame bases
(Ep^T Ep = Et^T Et = I), padded to K=32 for partition alignment:
    err2[i,j] = |x_i|^2 + |y_i|^2 + |d_j|^2 - 2 x^T M y - 2 x.a + 2 y.b + eps
    M = Ep^T Et (9), a = Ep^T d (3), b = Et^T d (3), d = Ep.op - Et.ot
    u rows (i side)  = [-2 x(x)y | -2x | 2y | s | 1 | 1 | 0pad]
    G rows (j side)  = [M9 | a | b | 1 | |d|^2 | eps | 0pad]
    out = sqrt(PSUM)  (eps rides inside the matmul)

Frame bases via the bisector identity (one reciprocal instead of two):
    u = |c'|a' + |a'|c',  v = |a'|c' - |c'|a'   (a' = a-b, c' = c-b)
    e1 = u/|u|, e2 = v/|v|, e3 = (u x v) / (|u||v|)

Layout: frames DMA with partition stride 9 so partition p holds j = p+128c
(natural order) -> all PSUM->SBUF copies after PE transposes are contiguous.
Output in bf16 (rel tolerance 2e-2; halves HBM write traffic).

Sharding: output rows i split across 8 cores; frame prep replicated.
"""

import numpy as np

B = 2            # batches
N = 2048         # n residues
NCORES = 8
RPC = N // NCORES          # rows per core per batch = 256
P = 128                    # partitions
NCH = N // P               # j-chunks per batch = 16
BC = B * NCH               # (b, c) groups = 32
G64 = 2 * BC               # (t, b, c) groups = 64
K = 32                     # padded contraction dim (18 used)
EPS_ERR = 1e-8
EPS_N2 = 1e-16             # under-sqrt clamp for norms (~(1e-8)^2)

_cache = {}


def _build(mm_f32r=False):
    import concourse.bass as bass
    import concourse.bacc as bacc
    import concourse.tile as tile
    import concourse.mybir as mybir
    from concourse.masks import make_identity

    F32 = mybir.dt.float32
    F32R = mybir.dt.float32r
    BF16 = mybir.dt.bfloat16
    MUL = mybir.AluOpType.mult
    ADD = mybir.AluOpType.add
    SUB = mybir.AluOpType.subtract
    SQRT = mybir.ActivationFunctionType.Sqrt
    AXX = mybir.AxisListType.X

    nc = bacc.Bacc("TRN2", target_bir_lowering=False, debug=False,
                   num_devices=NCORES)

    pc_d = nc.dram_tensor("pc", [B, RPC, 3], F32, kind="ExternalInput")
    tc_d = nc.dram_tensor("tcrd", [B, RPC, 3], F32, kind="ExternalInput")
    pf_d = nc.dram_tensor("pf", [B, N, 3, 3], F32, kind="ExternalInput")
    tf_d = nc.dram_tensor("tf", [B, N, 3, 3], F32, kind="ExternalInput")
    out_d = nc.dram_tensor("out", [B, RPC, N], BF16, kind="ExternalOutput")

    def v(tileap, offset_elems, dims):
        """AP view: keep partition dim of `tileap`, custom free dims."""
        return bass.AP(tensor=tileap.tensor,
                       offset=tileap.offset + offset_elems,
                       ap=[tileap.ap[0]] + dims)

    MMDT = F32R if mm_f32r else F32

    with tile.TileContext(nc) as tc:
        with (
            tc.tile_pool(name="consts", bufs=1) as consts,
            tc.tile_pool(name="prep", bufs=1) as prep,
            tc.tile_pool(name="gtp", bufs=1) as gtp,
            tc.tile_pool(name="ps_t", bufs=2, space="PSUM") as ps_t,
            tc.tile_pool(name="ps_mm", bufs=4, space="PSUM") as ps_mm,
            tc.tile_pool(name="outp", bufs=4) as outp,
        ):
            ident = consts.tile([P, P], F32)
            make_identity(nc, ident[:])
            epsn_t = consts.tile([P, 1], F32)
            nc.vector.memset(epsn_t[:], EPS_N2)
            eps_t = consts.tile([P, 1], F32)
            nc.vector.memset(eps_t[:], EPS_ERR)

            # ================= input DMAs =================
            # frames: partition p <- j = p + 128c  (partition stride 9).
            # Strided descriptors are expensive to issue (~1.5us each), so
            # spread the issues across four engine queues in parallel.
            F = prep.tile([P, 2, B, NCH, 9], F32, name="F")
            dma_engs = [nc.sync, nc.scalar, nc.gpsimd]
            k_ = 0
            for t, dram in enumerate((pf_d, tf_d)):
                for b in range(B):
                    src = bass.AP(tensor=dram, offset=b * N * 9,
                                  ap=[[9, P], [1152, NCH], [1, 9]])
                    dma_engs[k_ % 3].dma_start(out=F[:, t, b], in_=src)
                    k_ += 1

            # i-side inputs: MU [P, 4(b,r), 32]; x -> slots 9:12, y -> 12:15
            MU = prep.tile([P, B * 2, K], F32, name="MU")
            mu_ap = MU[:]
            for b in range(B):
                for t, dram in enumerate((pc_d, tc_d)):
                    src = bass.AP(tensor=dram, offset=b * RPC * 3,
                                  ap=[[3, P], [P * 3, 2], [1, 3]])
                    dst = v(mu_ap, (2 * b) * K + 9 + 3 * t,
                            [[K, 2], [1, 3]])
                    dma_engs[k_ % 3].dma_start(out=dst, in_=src)
                    k_ += 1

            # ================= constants / pads =================
            # u rows: [0:9 -2xy | 9:12 -2x | 12:15 2y | 15 '1' | 16:19 x^2 |
            #          19:22 2x-pairs | 22:25 y^2 | 25:28 2y-pairs | 28: 0]
            # G rows: [0:9 M | 9:12 a | 12:15 b | 15 |d|^2 | 16:19 EpTEp
            #          diag | 19:22 EpTEp off | 22:25 EtTEt diag | 25:28
            #          EtTEt off | 28: 0]; eps added via sqrt bias.
            nc.vector.memset(v(mu_ap, 15, [[K, 4], [1, 1]]), 1.0)
            nc.vector.memset(v(mu_ap, 28, [[K, 4], [1, K - 28]]), 0.0)
            G32 = prep.tile([P, BC, K], F32, name="G32")
            g_ap = G32[:]
            nc.gpsimd.memset(v(g_ap, 28, [[K, BC], [1, K - 28]]), 0.0)

            # ================= i-side (u rows) =================
            # xy9 = x (x) y -> slots 0:9   (reads raw slots 9:15)
            nc.gpsimd.tensor_tensor(
                out=v(mu_ap, 0, [[K, 4], [3, 3], [1, 3]]),
                in0=v(mu_ap, 9, [[K, 4], [1, 3], [0, 3]]),
                in1=v(mu_ap, 12, [[K, 4], [0, 3], [1, 3]]), op=MUL)
            # x^2 -> 16:19, y^2 -> 22:25 (raw coords)
            nc.gpsimd.tensor_tensor(
                out=v(mu_ap, 16, [[K, 4], [1, 3]]),
                in0=v(mu_ap, 9, [[K, 4], [1, 3]]),
                in1=v(mu_ap, 9, [[K, 4], [1, 3]]), op=MUL)
            nc.gpsimd.tensor_tensor(
                out=v(mu_ap, 22, [[K, 4], [1, 3]]),
                in0=v(mu_ap, 12, [[K, 4], [1, 3]]),
                in1=v(mu_ap, 12, [[K, 4], [1, 3]]), op=MUL)
            # pair products: (01),(12) -> 19,21 / 25,27 ; (02) -> 20 / 26
            for base, cs in ((19, 9), (25, 12)):
                nc.gpsimd.tensor_tensor(
                    out=v(mu_ap, base, [[K, 4], [2, 2]]),
                    in0=v(mu_ap, cs, [[K, 4], [1, 2]]),
                    in1=v(mu_ap, cs + 1, [[K, 4], [1, 2]]), op=MUL)
                nc.gpsimd.tensor_tensor(
                    out=v(mu_ap, base + 1, [[K, 4], [1, 1]]),
                    in0=v(mu_ap, cs, [[K, 4], [1, 1]]),
                    in1=v(mu_ap, cs + 2, [[K, 4], [1, 1]]), op=MUL)
            # scales: 0:12 *= -2 ; 12:15 *= 2 ; pair rows *= 2
            nc.gpsimd.tensor_scalar_mul(
                v(mu_ap, 0, [[K, 4], [1, 12]]),
                v(mu_ap, 0, [[K, 4], [1, 12]]), -2.0)
            nc.gpsimd.tensor_scalar_mul(
                v(mu_ap, 12, [[K, 4], [1, 3]]),
                v(mu_ap, 12, [[K, 4], [1, 3]]), 2.0)
            nc.gpsimd.tensor_scalar_mul(
                v(mu_ap, 19, [[K, 4], [6, 2], [1, 3]]),
                v(mu_ap, 19, [[K, 4], [6, 2], [1, 3]]), 2.0)
            # transpose -> four MT tiles [32, 128], all partition-base 0
            # (matmul requires lhsT and rhs to share the same base)
            MT = []
            for b in range(B):
                tp_m = ps_t.tile([64, P], F32, name=f"tp_m{b}", tag="tp")
                nc.tensor.transpose(tp_m[:], MU[:, 2 * b:2 * b + 2, :],
                                    ident[:])
                for r in range(2):
                    MTbr = prep.tile([K, P], MMDT, name=f"MT{b}{r}")
                    nc.vector.tensor_copy(out=MTbr[:],
                                          in_=tp_m[32 * r:32 * r + K, :])
                    MT.append(MTbr)

            # ================= G side: frame bases =================
            fap = F[:]
            def fpt(pt, extra):
                return v(fap, pt, [[9, G64]] + extra)

            # W = [a-b (w=0), c-b (w=1)]  [P, 2, G64, 3]
            W = prep.tile([P, 2, G64, 3], F32, name="W")
            nc.vector.tensor_tensor(
                out=W[:],
                in0=v(fap, 0, [[2, 2], [9, G64], [3, 3]]),
                in1=v(fap, 1, [[0, 2], [9, G64], [3, 3]]), op=SUB)
            SQW = prep.tile([P, 2, G64, 3], F32, name="SQW")
            nc.vector.tensor_tensor(out=SQW[:], in0=W[:], in1=W[:], op=MUL)
            SSW = prep.tile([P, 2, G64], F32, name="SSW")
            nc.vector.tensor_reduce(out=SSW[:], in_=SQW[:], axis=AXX, op=ADD)
            NRM = prep.tile([P, 2, G64], F32, name="NRM")
            nc.scalar.activation(out=NRM[:], in_=SSW[:], func=SQRT,
                                 bias=epsn_t[:], scale=1.0)
            # P1[0] = |c'| * a', P1[1] = |a'| * c'
            P1 = prep.tile([P, 2, G64, 3], F32, name="P1")
            p1_ap = P1[:]
            nrm_ap = NRM[:]
            w_ap = W[:]
            nc.vector.tensor_tensor(
                out=v(p1_ap, 0, [[3, G64], [1, 3]]),
                in0=v(w_ap, 0, [[3, G64], [1, 3]]),
                in1=v(nrm_ap, G64, [[1, G64], [0, 3]]), op=MUL)
            nc.vector.tensor_tensor(
                out=v(p1_ap, G64 * 3, [[3, G64], [1, 3]]),
                in0=v(w_ap, G64 * 3, [[3, G64], [1, 3]]),
                in1=v(nrm_ap, 0, [[1, G64], [0, 3]]), op=MUL)
            # UV[0] = u = P1[0]+P1[1], UV[1] = v = P1[1]-P1[0]
            UV = prep.tile([P, 2, G64, 3], F32, name="UV")
            uv_ap = UV[:]
            nc.vector.tensor_tensor(
                out=v(uv_ap, 0, [[3, G64], [1, 3]]),
                in0=v(p1_ap, 0, [[3, G64], [1, 3]]),
                in1=v(p1_ap, G64 * 3, [[3, G64], [1, 3]]), op=ADD)
            nc.vector.tensor_tensor(
                out=v(uv_ap, G64 * 3, [[3, G64], [1, 3]]),
                in0=v(p1_ap, G64 * 3, [[3, G64], [1, 3]]),
                in1=v(p1_ap, 0, [[3, G64], [1, 3]]), op=SUB)
            # norms of u, v
            SQ2 = prep.tile([P, 2, G64, 3], F32, name="SQ2")
            nc.vector.tensor_tensor(out=SQ2[:], in0=UV[:], in1=UV[:], op=MUL)
            SS2 = prep.tile([P, 2, G64], F32, name="SS2")
            nc.vector.tensor_reduce(out=SS2[:], in_=SQ2[:], axis=AXX, op=ADD)
            NRM2 = prep.tile([P, 2, G64], F32, name="NRM2")
            nc.scalar.activation(out=NRM2[:], in_=SS2[:], func=SQRT,
                                 bias=epsn_t[:], scale=1.0)
            RCP = prep.tile([P, 2, G64], F32, name="RCP")
            nc.vector.reciprocal(RCP[:], NRM2[:])
            rcp_ap = RCP[:]

            # cross product path (parallel to normalize): CR = u x v
            # per-component muls with rotated views (no dup copies)
            TA = prep.tile([P, G64, 3], F32, name="TA")
            TB = prep.tile([P, G64, 3], F32, name="TB")
            for x in range(3):
                yy, zz = (x + 1) % 3, (x + 2) % 3
                nc.gpsimd.tensor_tensor(
                    out=v(TA[:], x, [[3, G64]]),
                    in0=v(uv_ap, yy, [[3, G64]]),
                    in1=v(uv_ap, G64 * 3 + zz, [[3, G64]]), op=MUL)
                nc.vector.tensor_tensor(
                    out=v(TB[:], x, [[3, G64]]),
                    in0=v(uv_ap, zz, [[3, G64]]),
                    in1=v(uv_ap, G64 * 3 + yy, [[3, G64]]), op=MUL)
            CR = prep.tile([P, G64, 3], F32, name="CR")
            nc.vector.tensor_tensor(out=CR[:], in0=TA[:], in1=TB[:], op=SUB)
            RR = prep.tile([P, G64], F32, name="RR")
            nc.gpsimd.tensor_tensor(
                out=RR[:], in0=v(rcp_ap, 0, [[1, G64]]),
                in1=v(rcp_ap, G64, [[1, G64]]), op=MUL)

            # EE [P, G64, 3k, 3d]: e1 = u*rcp_u, e2 = v*rcp_v, e3 = CR*RR
            EE = prep.tile([P, G64, 3, 3], F32, name="EE")
            ee_ap = EE[:]
            nc.vector.tensor_tensor(
                out=v(ee_ap, 0, [[3, 2], [9, G64], [1, 3]]),
                in0=v(uv_ap, 0, [[G64 * 3, 2], [3, G64], [1, 3]]),
                in1=v(rcp_ap, 0, [[G64, 2], [1, G64], [0, 3]]), op=MUL)
            nc.vector.tensor_tensor(
                out=v(ee_ap, 6, [[9, G64], [1, 3]]),
                in0=CR[:],
                in1=v(RR[:], 0, [[1, G64], [0, 3]]), op=MUL)

            # origin projections: OC[g,k] = sum_d EE[g,k,d]*orig[g,d]
            OP = prep.tile([P, G64, 3, 3], F32, name="OP")
            nc.vector.tensor_tensor(
                out=OP[:], in0=EE[:],
                in1=v(fap, 1, [[9, G64], [0, 3], [3, 3]]), op=MUL)
            OC = prep.tile([P, G64, 3], F32, name="OC")
            nc.vector.tensor_reduce(out=OC[:], in_=OP[:], axis=AXX, op=ADD)
            # d = cp - ct  [P, BC, 3]
            D = prep.tile([P, BC, 3], F32, name="D")
            nc.vector.tensor_tensor(
                out=D[:], in0=v(OC[:], 0, [[3, BC], [1, 3]]),
                in1=v(OC[:], BC * 3, [[3, BC], [1, 3]]), op=SUB)

            # M[p,q] = sum_k Ep[k,p] Et[k,q] -> G slots 3p+q
            for p in range(3):
                TM = prep.tile([P, BC, 3, 3], F32, name=f"TM{p}", tag="tm")
                nc.vector.tensor_tensor(
                    out=TM[:],
                    in0=v(ee_ap, p, [[9, BC], [0, 3], [3, 3]]),
                    in1=v(ee_ap, BC * 9, [[9, BC], [1, 3], [3, 3]]), op=MUL)
                nc.vector.tensor_reduce(
                    out=v(g_ap, 3 * p, [[K, BC], [1, 3]]), in_=TM[:],
                    axis=AXX, op=ADD)
            # a = Ep^T d -> slots 9:12 ; b = Et^T d -> slots 12:15
            AB = prep.tile([P, 2, BC, 3, 3], F32, name="AB")
            for t in range(2):
                nc.vector.tensor_tensor(
                    out=AB[:, t],
                    in0=v(ee_ap, t * BC * 9 + 0, [[9, BC], [1, 3], [3, 3]]),
                    in1=v(D[:], 0, [[3, BC], [0, 3], [1, 3]]), op=MUL)
                nc.vector.tensor_reduce(
                    out=v(g_ap, 9 + 3 * t, [[K, BC], [1, 3]]), in_=AB[:, t],
                    axis=AXX, op=ADD)
            # |d|^2 -> slot 15
            DD = prep.tile([P, BC, 3], F32, name="DD")
            nc.gpsimd.tensor_tensor(out=DD[:], in0=D[:], in1=D[:], op=MUL)
            nc.vector.tensor_reduce(
                out=v(g_ap, 15, [[K, BC]]), in_=DD[:],
                axis=AXX, op=ADD)
            # E^T E rows (fp32 orthonormality defect matters at the rel
            # tolerance): diag -> 16:19 / 22:25, off (01),(12) -> 19,21 /
            # 25,27 stride 2, (02) -> 20 / 26
            for t in range(2):
                toff = t * BC * 9
                gd = 16 + 6 * t
                TD = prep.tile([P, BC, 3, 3], F32, name=f"TD{t}", tag="td")
                nc.gpsimd.tensor_tensor(
                    out=TD[:],
                    in0=v(ee_ap, toff, [[9, BC], [1, 3], [3, 3]]),
                    in1=v(ee_ap, toff, [[9, BC], [1, 3], [3, 3]]), op=MUL)
                nc.vector.tensor_reduce(
                    out=v(g_ap, gd, [[K, BC], [1, 3]]), in_=TD[:],
                    axis=AXX, op=ADD)
                TO2 = prep.tile([P, BC, 2, 3], F32, name=f"TO2{t}", tag="to2")
                nc.gpsimd.tensor_tensor(
                    out=TO2[:],
                    in0=v(ee_ap, toff, [[9, BC], [1, 2], [3, 3]]),
                    in1=v(ee_ap, toff + 1, [[9, BC], [1, 2], [3, 3]]),
                    op=MUL)
                nc.vector.tensor_reduce(
                    out=v(g_ap, gd + 3, [[K, BC], [2, 2]]), in_=TO2[:],
                    axis=AXX, op=ADD)
                TO1 = prep.tile([P, BC, 1, 3], F32, name=f"TO1{t}", tag="to1")
                nc.gpsimd.tensor_tensor(
                    out=TO1[:],
                    in0=v(ee_ap, toff, [[9, BC], [1, 1], [3, 3]]),
                    in1=v(ee_ap, toff + 2, [[9, BC], [1, 1], [3, 3]]),
                    op=MUL)
                nc.vector.tensor_reduce(
                    out=v(g_ap, gd + 4, [[K, BC], [1, 1]]), in_=TO1[:],
                    axis=AXX, op=ADD)

            # ================= transposes + matmuls =================
            GT = {}
            for b in range(B):
                for g in range(4):
                    GT[(b, g)] = gtp.tile([K, 512], MMDT, name=f"gt{b}{g}",
                                          tag=f"gt{b}{g}")
            for b in range(B):
                for g in range(4):
                    for h in range(2):
                        # 2 chunks x 32 slots = 64 free
                        tp = ps_t.tile([64, P], F32, name=f"tpg{b}{g}{h}",
                                       tag="tp")
                        nc.tensor.transpose(
                            tp[:],
                            v(g_ap, (b * NCH + 4 * g + 2 * h) * K, [[1, 64]]),
                            ident[:])
                        for c in range(2):
                            nc.vector.tensor_copy(
                                out=GT[(b, g)][:, (2 * h + c) * P:
                                               (2 * h + c + 1) * P],
                                in_=tp[32 * c:32 * c + K, :])

            for b in range(B):
                for r in range(2):
                    br = 2 * b + r
                    OT = outp.tile([P, N], BF16, name=f"ot{br}", tag="ot")
                    for g in range(4):
                        mm = ps_mm.tile([P, 512], F32, name=f"mm{br}{g}",
                                        tag="mm")
                        nc.tensor.matmul(
                            mm[:], MT[br][:], GT[(b, g)][:],
                            start=True, stop=True)
                        nc.scalar.activation(
                            out=OT[:, g * 512:(g + 1) * 512], in_=mm[:],
                            func=SQRT, bias=eps_t[:], scale=1.0)
                    for h in range(2):
                        dst = bass.AP(
                            tensor=out_d,
                            offset=(b * RPC + r * P) * N + h * 1024,
                            ap=[[N, P], [1, 1024]])
                        nc.sync.dma_start(out=dst, in_=OT[:, h * 1024:
                                                          (h + 1) * 1024])

    nc.compile()
    return nc


def _get_nc():
    if "nc" not in _cache:
        _cache["nc"] = _build()
    return _cache["nc"]


def _in_maps(pred_coords, true_coords, pred_frames, true_frames):
    pc = np.ascontiguousarray(pred_coords, dtype=np.float32)
    tcd = np.ascontiguousarray(true_coords, dtype=np.float32)
    pf = np.ascontiguousarray(pred_frames, dtype=np.float32)
    tf = np.ascontiguousarray(true_frames, dtype=np.float32)
    maps = []
    for c in range(NCORES):
        sl = slice(c * RPC, (c + 1) * RPC)
        maps.append({
            "pc": np.ascontiguousarray(pc[:, sl]),
            "tcrd": np.ascontiguousarray(tcd[:, sl]),
            "pf": pf,
            "tf": tf,
        })
    return maps


def _assemble(results):
    full = np.empty((B, N, N), dtype=np.float32)
    for c in range(NCORES):
        full[:, c * RPC:(c + 1) * RPC, :] = np.asarray(
            results[c]["out"]).astype(np.float32)
    return full


def run_hw(trace=False, **inputs):
    from concourse.bass_utils import run_bass_kernel_spmd
    nc = _get_nc()
    res = run_bass_kernel_spmd(nc, _in_maps(**inputs), list(range(NCORES)),
                               trace=trace)
    return _assemble(res.results), res


def kernel(**inputs):
    out, _ = run_hw(trace=False, **inputs)
    return out
